# revision 17
# baseline (speedup 1.0000x reference)
"""Trainium2 Bass kernel for an AttentionBlock (B=16, C=256, N=1024 tokens,
4 heads x d_k=64), data-parallel over batch across 8 NeuronCores.

Layout strategy: all device math runs in "transposed" token-last layout.
x[b] arrives as [C, N] which is exactly xf^T, the natural stationary operand
(lhsT) for every matmul, and the output y^T [C, N] is exactly the layout the
problem wants back ([B, C, H, W]).  No transposes anywhere.

Per batch element, per core:
  qk^T [512, N] = W_qk^T @ xf^T     (q pre-scaled by dk^-0.5, +b_q on copy;
                                     b_k dropped: constant-over-keys terms
                                     cancel in softmax)
  v    [N, 4, 128]                  (tokens on partitions; cols 64..127 = 1.0
                                     so the PV matmul emits the softmax
                                     denominator replicated on rows 64..127)
  S^T  [j, i] per head = k^T.T @ q^T  -- two d_k=64 heads packed in the
                                     128-row PE array via row strips
  P^T = exp(S^T)                    (no max subtraction: |scores| <= ~11,
                                     exp <= ~1.4e4, safe in fp32)
  O^T[128, i] = v_aug.T @ P^T       accumulated over 8 j-tiles; rows 0-63 are
                                     unnormalized O^T, rows 64-127 all carry
                                     the denominator -> 64-lane reciprocal +
                                     one multiply normalizes, no PE involved
  y^T = W_out^T @ res^T + (b_out + b_v @ W_out) + x^T

All matmuls run as float32r (full fp32 storage, 1 cycle/row on the PE vs 4
for plain fp32); producers feeding matmuls tag outputs f32r via bitcast to
satisfy the BIR verifier.
"""

import numpy as np

N_CORES = 8
B, C = 16, 256
N = 1024  # H*W = 32*32
NH, DK = 4, 64
BPC = B // N_CORES  # batch elements per core
P = 128
KT = 2  # C / 128 contraction tiles
ISZ = 512  # i-tile (query) width
NI = N // ISZ  # 2
NJ = N // P  # 8 key tiles
SCALE = DK ** -0.5

_CACHE = {}


def _build_module():
    import concourse.bass as bass  # noqa: F401
    import concourse.mybir as mybir
    import concourse.tile as tile
    from concourse import bacc

    f32 = mybir.dt.float32
    f32r = mybir.dt.float32r
    ADD = mybir.AluOpType.add
    EXP = mybir.ActivationFunctionType.Exp

    nc = bacc.Bacc(
        "TRN2",
        debug=False,
        enable_asserts=False,
        target_bir_lowering=False,
        num_devices=N_CORES,
    )

    x_d = nc.dram_tensor("x", [BPC, C, N], f32, kind="ExternalInput").ap()
    wqk_d = nc.dram_tensor("wqk", [C, 4 * P], f32, kind="ExternalInput").ap()
    wv_d = nc.dram_tensor("wv", [C, NH * DK], f32, kind="ExternalInput").ap()
    wout_d = nc.dram_tensor("wout", [C, C], f32, kind="ExternalInput").ap()
    bq_d = nc.dram_tensor("bq", [P, 2], f32, kind="ExternalInput").ap()
    beff_d = nc.dram_tensor("beff", [P, 2], f32, kind="ExternalInput").ap()
    ones_d = nc.dram_tensor("ones", [P, NJ * NH * DK], f32, kind="ExternalInput").ap()
    y_d = nc.dram_tensor("y", [BPC, C, N], f32, kind="ExternalOutput").ap()

    with tile.TileContext(nc) as tc:
        with (
            tc.tile_pool(name="const", bufs=1) as const,
            tc.tile_pool(name="xp", bufs=2) as xp,
            tc.tile_pool(name="qkp", bufs=2) as qkp,
            tc.tile_pool(name="vp", bufs=1) as vp,
            tc.tile_pool(name="ptp", bufs=6) as ptp,
            tc.tile_pool(name="resp", bufs=2) as resp,
            tc.tile_pool(name="smp", bufs=4) as smp,
            tc.tile_pool(name="outp", bufs=4) as outp,
            tc.tile_pool(name="psg", bufs=2, space="PSUM") as psg,
            tc.tile_pool(name="pss", bufs=2, space="PSUM") as pss,
            tc.tile_pool(name="psv", bufs=1, space="PSUM") as psv,
        ):
            # --- constants (wqk first: it gates the first matmuls) ---
            wqk_sb = const.tile([P, KT, 4 * P], f32, tag="wqk")
            nc.sync.dma_start(
                wqk_sb.bitcast(f32r),
                wqk_d.rearrange("(kt p) m -> p kt m", p=P).bitcast(f32r),
            )
            wv_sb = const.tile([P, KT, NH * DK], f32, tag="wv")
            nc.sync.dma_start(
                wv_sb.bitcast(f32r),
                wv_d.rearrange("(kt p) m -> p kt m", p=P).bitcast(f32r),
            )
            wout_sb = const.tile([P, KT, C], f32, tag="wout")
            nc.sync.dma_start(
                wout_sb.bitcast(f32r),
                wout_d.rearrange("(kt p) m -> p kt m", p=P).bitcast(f32r),
            )
            bq_sb = const.tile([P, 2], f32, tag="bq")
            nc.sync.dma_start(bq_sb, bq_d)
            beff_sb = const.tile([P, 2], f32, tag="beff")
            nc.sync.dma_start(beff_sb, beff_d)

            # persistent v tiles (one per batch element); ones block
            # [*, jt, h, 64:128] written once via DMA
            v_tiles = []
            for vb in range(BPC):
                v_sb = vp.tile([P, NJ, NH, 2 * DK], f32, tag=f"v{vb}", name=f"v{vb}")
                nc.vector.dma_start(
                    v_sb[:, :, :, DK:].bitcast(f32r),
                    ones_d.rearrange("p (j h d) -> p j h d", j=NJ, h=NH).bitcast(
                        f32r
                    ),
                )
                v_tiles.append(v_sb)

            for b in range(BPC):
                x_sb = xp.tile([P, KT, N], f32, tag="x")
                # split in halves so the first qk/v matmuls start sooner
                for half in range(2):
                    sl = slice(half * (N // 2), (half + 1) * (N // 2))
                    nc.scalar.dma_start(
                        x_sb[:, :, sl].bitcast(f32r),
                        x_d[b]
                        .rearrange("(kt p) n -> p kt n", p=P)[:, :, sl]
                        .bitcast(f32r),
                    )

                # --- qk generation: feature tiles [q01, k01, q23, k23] ---
                qk_sb = []
                for t in range(4):
                    qt = qkp.tile([P, N], f32, tag=f"qk{t}")
                    for i in range(NI):
                        ps = psg.tile([P, ISZ], f32, tag="gen")
                        for kt in range(KT):
                            nc.tensor.matmul(
                                ps,
                                lhsT=wqk_sb[:, kt, t * P : (t + 1) * P].bitcast(f32r),
                                rhs=x_sb[:, kt, i * ISZ : (i + 1) * ISZ].bitcast(f32r),
                                start=(kt == 0),
                                stop=(kt == KT - 1),
                            )
                        dst = qt[:, i * ISZ : (i + 1) * ISZ]
                        if t % 2 == 0:  # q tile: add pre-scaled bias
                            hp = t // 2
                            nc.vector.tensor_scalar_add(
                                dst.bitcast(f32r), ps, bq_sb[:, hp : hp + 1]
                            )
                        else:
                            nc.vector.tensor_copy(dst.bitcast(f32r), ps)
                    qk_sb.append(qt)

                # --- v generation into cols 0..63 ---
                v_sb = v_tiles[b]
                for jt in range(NJ):
                    ps = psg.tile([P, ISZ], f32, tag="gen")
                    psv_view = ps[:, : NH * DK]
                    for kt in range(KT):
                        nc.tensor.matmul(
                            psv_view,
                            lhsT=x_sb[:, kt, jt * P : (jt + 1) * P].bitcast(f32r),
                            rhs=wv_sb[:, kt, :].bitcast(f32r),
                            start=(kt == 0),
                            stop=(kt == KT - 1),
                        )
                    nc.vector.tensor_copy(
                        v_sb[:, jt, :, 0:DK].bitcast(f32r),
                        psv_view.rearrange("p (h d) -> p h d", h=NH),
                    )

                # --- attention; i outer so out-proj for i=0 overlaps i=1 ---
                res_sb = resp.tile([P, KT, N], f32, tag="res")
                for i in range(NI):
                    for hp in range(2):
                        q_t = qk_sb[2 * hp]
                        k_t = qk_sb[2 * hp + 1]
                        pvs = [
                            psv.tile([P, ISZ], f32, tag=f"pv{h}", name=f"pv{h}")
                            for h in range(2)
                        ]
                        for jt in range(NJ):
                            st = pss.tile([P, 2, ISZ], f32, tag="st")
                            for h in range(2):
                                nc.tensor.matmul(
                                    st[:, h, :],
                                    lhsT=k_t[
                                        h * DK : (h + 1) * DK, jt * P : (jt + 1) * P
                                    ].bitcast(f32r),
                                    rhs=q_t[
                                        h * DK : (h + 1) * DK, i * ISZ : (i + 1) * ISZ
                                    ].bitcast(f32r),
                                )
                            pt = ptp.tile([P, 2, ISZ], f32, tag="pt")
                            nc.scalar.activation(pt.bitcast(f32r), st, EXP)
                            for h in range(2):
                                nc.tensor.matmul(
                                    pvs[h],
                                    lhsT=v_sb[:, jt, 2 * hp + h, :].bitcast(f32r),
                                    rhs=pt[:, h, :].bitcast(f32r),
                                    start=(jt == 0),
                                    stop=(jt == NJ - 1),
                                )
                        # normalize: denominator is replicated on rows 64..127
                        for h in range(2):
                            rcp = smp.tile(
                                [DK, ISZ], f32, tag=f"rcp{h}", name=f"rcp{h}"
                            )
                            nc.vector.reciprocal(rcp, pvs[h][DK : 2 * DK, :])
                            nc.vector.tensor_mul(
                                res_sb[
                                    h * DK : (h + 1) * DK, hp, i * ISZ : (i + 1) * ISZ
                                ].bitcast(f32r),
                                pvs[h][0:DK, :],
                                rcp,
                            )

                    # --- output projection + bias + residual for this i ---
                    for mt in range(KT):
                        ps = psg.tile([P, ISZ], f32, tag="gen")
                        for kt in range(KT):
                            nc.tensor.matmul(
                                ps,
                                lhsT=wout_sb[:, kt, mt * P : (mt + 1) * P].bitcast(
                                    f32r
                                ),
                                rhs=res_sb[:, kt, i * ISZ : (i + 1) * ISZ].bitcast(
                                    f32r
                                ),
                                start=(kt == 0),
                                stop=(kt == KT - 1),
                            )
                        y_sb = outp.tile([P, ISZ], f32, tag="y")
                        nc.vector.scalar_tensor_tensor(
                            out=y_sb,
                            in0=ps,
                            scalar=beff_sb[:, mt : mt + 1],
                            in1=x_sb[:, mt, i * ISZ : (i + 1) * ISZ],
                            op0=ADD,
                            op1=ADD,
                        )
                        nc.sync.dma_start(
                            y_d[
                                b,
                                mt * P : (mt + 1) * P,
                                i * ISZ : (i + 1) * ISZ,
                            ],
                            y_sb,
                        )

    nc.compile()
    return nc


def _prep_weights(W_qkv, b_qkv, W_out, b_out):
    """Host-side weight reshuffles (cheap, [256, 768]-sized)."""
    Wr = np.ascontiguousarray(W_qkv, dtype=np.float32).reshape(C, NH, 3, DK)
    br = np.ascontiguousarray(b_qkv, dtype=np.float32).reshape(NH, 3, DK)
    # feature tiles: [q0|q1], [k0|k1], [q2|q3], [k2|k3]; q pre-scaled
    cols = []
    for hp in range(2):
        cols.append(Wr[:, 2 * hp, 0] * SCALE)
        cols.append(Wr[:, 2 * hp + 1, 0] * SCALE)
        cols.append(Wr[:, 2 * hp, 1])
        cols.append(Wr[:, 2 * hp + 1, 1])
    wqk = np.concatenate(cols, axis=1)  # [C, 512]
    bq = np.stack(
        [
            np.concatenate([br[2 * hp, 0], br[2 * hp + 1, 0]]) * SCALE
            for hp in range(2)
        ],
        axis=1,
    )  # [128, 2]
    wv = np.concatenate([Wr[:, h, 2] for h in range(NH)], axis=1)  # [C, 256]
    bv = np.concatenate([br[h, 2] for h in range(NH)])  # [256]
    W_out = np.ascontiguousarray(W_out, dtype=np.float32)
    b_eff = (b_out + bv @ W_out).astype(np.float32)  # [256]
    beff = b_eff.reshape(KT, P).T.copy()  # [128, 2] col=mt
    return (
        np.ascontiguousarray(wqk, dtype=np.float32),
        np.ascontiguousarray(bq, dtype=np.float32),
        np.ascontiguousarray(wv, dtype=np.float32),
        W_out,
        np.ascontiguousarray(beff, dtype=np.float32),
    )


def kernel(x, W_qkv, b_qkv, W_out, b_out):
    from concourse.bass_utils import run_bass_kernel_spmd

    if "nc" not in _CACHE:
        _CACHE["nc"] = _build_module()
    nc = _CACHE["nc"]

    x = np.ascontiguousarray(np.asarray(x), dtype=np.float32)
    Bx, Cx, Hx, Wx = x.shape
    x3 = x.reshape(Bx, Cx, Hx * Wx)
    wqk, bq, wv, wout, beff = _prep_weights(
        np.asarray(W_qkv), np.asarray(b_qkv), np.asarray(W_out), np.asarray(b_out)
    )

    ones = np.ones((P, NJ * NH * DK), dtype=np.float32)
    in_maps = []
    for c in range(N_CORES):
        in_maps.append(
            {
                "x": np.ascontiguousarray(x3[c * BPC : (c + 1) * BPC]),
                "wqk": wqk,
                "wv": wv,
                "wout": wout,
                "bq": bq,
                "beff": beff,
                "ones": ones,
            }
        )

    res = run_bass_kernel_spmd(nc, in_maps, core_ids=list(range(N_CORES)))
    y = np.concatenate([r["y"] for r in res.results], axis=0)  # [16, 256, 1024]
    return y.reshape(Bx, Cx, Hx, Wx).astype(np.float32)


# revision 32
# speedup vs baseline: 48.3985x; 48.3985x over previous
"""Trainium2 Bass kernel for an AttentionBlock (B=16, C=256, N=1024 tokens,
4 heads x d_k=64), data-parallel over batch across 8 NeuronCores.

Layout strategy: all device math runs in "transposed" token-last layout.
x[b] arrives as [C, N] which is exactly xf^T, the natural stationary operand
(lhsT) for every matmul, and the output y^T [C, N] is exactly the layout the
problem wants back ([B, C, H, W]).  No transposes anywhere.

Per batch element, per core:
  qk^T [512, N] = W_qk^T @ xf^T     (q pre-scaled by dk^-0.5, +b_q on copy;
                                     b_k dropped: constant-over-keys terms
                                     cancel in softmax)
  v    [N, 4, 128]                  (tokens on partitions; cols 64..127 = 1.0
                                     so the PV matmul emits the softmax
                                     denominator replicated on rows 64..127)
  S^T  [j, i] per head = k^T.T @ q^T  -- two d_k=64 heads packed in the
                                     128-row PE array via row strips
  P^T = exp(S^T)                    (no max subtraction: |scores| <= ~11,
                                     exp <= ~1.4e4, safe in fp32)
  O^T[128, i] = v_aug.T @ P^T       accumulated over 8 j-tiles; rows 0-63 are
                                     unnormalized O^T, rows 64-127 all carry
                                     the denominator -> 64-lane reciprocal +
                                     one multiply normalizes, no PE involved
  y^T = W_out^T @ res^T + (b_out + b_v @ W_out) + x^T

All matmuls run as float32r (full fp32 storage, 1 cycle/row on the PE vs 4
for plain fp32); producers feeding matmuls tag outputs f32r via bitcast to
satisfy the BIR verifier.
"""

import numpy as np

N_CORES = 8
B, C = 16, 256
N = 1024  # H*W = 32*32
NH, DK = 4, 64
BPC = B // N_CORES  # batch elements per core
P = 128
KT = 2  # C / 128 contraction tiles
ISZ = 512  # i-tile (query) width
NI = N // ISZ  # 2
NJ = N // P  # 8 key tiles
SCALE = DK ** -0.5

_CACHE = {}


def _build_module():
    import concourse.bass as bass  # noqa: F401
    import concourse.mybir as mybir
    import concourse.tile as tile
    from concourse import bacc

    f32 = mybir.dt.float32
    f32r = mybir.dt.float32r
    ADD = mybir.AluOpType.add
    EXP = mybir.ActivationFunctionType.Exp

    nc = bacc.Bacc(
        "TRN2",
        debug=False,
        enable_asserts=False,
        target_bir_lowering=False,
        num_devices=N_CORES,
    )

    x_d = nc.dram_tensor("x", [BPC, C, N], f32, kind="ExternalInput").ap()
    wqk_d = nc.dram_tensor("wqk", [C, 4 * P], f32, kind="ExternalInput").ap()
    wv_d = nc.dram_tensor("wv", [C, NH * DK], f32, kind="ExternalInput").ap()
    wout_d = nc.dram_tensor("wout", [C, C], f32, kind="ExternalInput").ap()
    bq_d = nc.dram_tensor("bq", [P, 2], f32, kind="ExternalInput").ap()
    beff_d = nc.dram_tensor("beff", [P, 2], f32, kind="ExternalInput").ap()
    ones_d = nc.dram_tensor("ones", [P, DK], f32, kind="ExternalInput").ap()
    y_d = nc.dram_tensor("y", [BPC, C, N], f32, kind="ExternalOutput").ap()

    with tile.TileContext(nc) as tc:
        with (
            tc.tile_pool(name="const", bufs=1) as const,
            tc.tile_pool(name="xp", bufs=2) as xp,
            tc.tile_pool(name="qkp", bufs=2) as qkp,
            tc.tile_pool(name="vp", bufs=1) as vp,
            tc.tile_pool(name="ptp", bufs=6) as ptp,
            tc.tile_pool(name="resp", bufs=2) as resp,
            tc.tile_pool(name="smp", bufs=4) as smp,
            tc.tile_pool(name="outp", bufs=4) as outp,
            tc.tile_pool(name="psg", bufs=2, space="PSUM") as psg,
            tc.tile_pool(name="pss", bufs=2, space="PSUM") as pss,
            tc.tile_pool(name="psv", bufs=2, space="PSUM") as psv,
        ):
            # --- constants: weights on the SP queue (wqk first, it gates
            # the first matmuls); small tensors on the idle gpsimd queue ---
            bq_sb = const.tile([P, 2], f32, tag="bq")
            nc.gpsimd.dma_start(bq_sb, bq_d)
            beff_sb = const.tile([P, 2], f32, tag="beff")
            nc.gpsimd.dma_start(beff_sb, beff_d)
            ones_sb = const.tile([P, DK], f32, tag="ones")
            nc.gpsimd.dma_start(ones_sb.bitcast(f32r), ones_d.bitcast(f32r))
            wqk_sb = const.tile([P, KT, 4 * P], f32, tag="wqk")
            nc.sync.dma_start(
                wqk_sb.bitcast(f32r),
                wqk_d.rearrange("(kt p) m -> p kt m", p=P).bitcast(f32r),
            )
            wv_sb = const.tile([P, KT, NH * DK], f32, tag="wv")
            nc.sync.dma_start(
                wv_sb.bitcast(f32r),
                wv_d.rearrange("(kt p) m -> p kt m", p=P).bitcast(f32r),
            )
            wout_sb = const.tile([P, KT, C], f32, tag="wout")
            nc.sync.dma_start(
                wout_sb.bitcast(f32r),
                wout_d.rearrange("(kt p) m -> p kt m", p=P).bitcast(f32r),
            )
            # persistent v tiles (one per batch element); ones block
            # [*, jt, h, 64:128] written once via a broadcast DVE copy
            v_tiles = [
                vp.tile([P, NJ, NH, 2 * DK], f32, tag=f"v{vb}", name=f"v{vb}")
                for vb in range(BPC)
            ]

            def emit_xload(b):
                x_sb = xp.tile([P, KT, N], f32, tag="x", name=f"x{b}")
                # split in halves so the first qk/v matmuls start sooner
                for half in range(2):
                    sl = slice(half * (N // 2), (half + 1) * (N // 2))
                    nc.scalar.dma_start(
                        x_sb[:, :, sl].bitcast(f32r),
                        x_d[b]
                        .rearrange("(kt p) n -> p kt n", p=P)[:, :, sl]
                        .bitcast(f32r),
                    )
                return x_sb

            def emit_qkgen(b, x_sb):
                # feature tiles [q01, k01, q23, k23]; i outer so the first
                # half of x unblocks all four tiles
                qk_sb = [
                    qkp.tile([P, N], f32, tag=f"qk{t}", name=f"qk{t}_{b}")
                    for t in range(4)
                ]
                for i in range(NI):
                    for t in range(4):
                        qt = qk_sb[t]
                        ps = psg.tile([P, ISZ], f32, tag="gen", name="gqk")
                        for kt in range(KT):
                            nc.tensor.matmul(
                                ps,
                                lhsT=wqk_sb[:, kt, t * P : (t + 1) * P].bitcast(f32r),
                                rhs=x_sb[:, kt, i * ISZ : (i + 1) * ISZ].bitcast(f32r),
                                start=(kt == 0),
                                stop=(kt == KT - 1),
                            )
                        dst = qt[:, i * ISZ : (i + 1) * ISZ]
                        if t % 2 == 0:  # q tile: add pre-scaled bias
                            hp = t // 2
                            nc.vector.tensor_scalar_add(
                                dst.bitcast(f32r), ps, bq_sb[:, hp : hp + 1]
                            )
                        else:
                            nc.vector.tensor_copy(dst.bitcast(f32r), ps)
                return qk_sb

            def emit_vgen(b, x_sb):
                v_sb = v_tiles[b]
                nc.vector.tensor_copy(
                    v_sb[:, :, :, DK:].bitcast(f32r),
                    ones_sb.rearrange("p (a b d) -> p a b d", a=1, b=1).to_broadcast(
                        [P, NJ, NH, DK]
                    ),
                )
                for jt in range(NJ):
                    ps = psg.tile([P, ISZ], f32, tag="gen", name="gv")
                    psv_view = ps[:, : NH * DK]
                    for kt in range(KT):
                        nc.tensor.matmul(
                            psv_view,
                            lhsT=x_sb[:, kt, jt * P : (jt + 1) * P].bitcast(f32r),
                            rhs=wv_sb[:, kt, :].bitcast(f32r),
                            start=(kt == 0),
                            stop=(kt == KT - 1),
                        )
                    nc.vector.tensor_copy(
                        v_sb[:, jt, :, 0:DK].bitcast(f32r),
                        psv_view.rearrange("p (h d) -> p h d", h=NH),
                    )
                return v_sb

            def emit_attn_group(i, hp, qk_sb, v_sb, res_sb):
                q_t = qk_sb[2 * hp]
                k_t = qk_sb[2 * hp + 1]
                pvs = [
                    psv.tile([P, ISZ], f32, tag="pv", name=f"pv{h}")
                    for h in range(2)
                ]
                for jt in range(NJ):
                    st = pss.tile([P, 2, ISZ], f32, tag="st", name="st")
                    for h in range(2):
                        nc.tensor.matmul(
                            st[:, h, :],
                            lhsT=k_t[
                                h * DK : (h + 1) * DK, jt * P : (jt + 1) * P
                            ].bitcast(f32r),
                            rhs=q_t[
                                h * DK : (h + 1) * DK, i * ISZ : (i + 1) * ISZ
                            ].bitcast(f32r),
                        )
                    pt = ptp.tile([P, 2, ISZ], f32, tag="pt", name="pt")
                    nc.scalar.activation(pt.bitcast(f32r), st, EXP)
                    for h in range(2):
                        nc.tensor.matmul(
                            pvs[h],
                            lhsT=v_sb[:, jt, 2 * hp + h, :].bitcast(f32r),
                            rhs=pt[:, h, :].bitcast(f32r),
                            start=(jt == 0),
                            stop=(jt == NJ - 1),
                        )
                # normalize: denominator replicated on rows 64..127
                for h in range(2):
                    rcp = smp.tile([DK, ISZ], f32, tag=f"rcp{h}", name=f"rcp{h}")
                    nc.vector.reciprocal(rcp, pvs[h][DK : 2 * DK, :])
                    nc.vector.tensor_mul(
                        res_sb[
                            h * DK : (h + 1) * DK, hp, i * ISZ : (i + 1) * ISZ
                        ].bitcast(f32r),
                        pvs[h][0:DK, :],
                        rcp,
                    )

            def emit_outproj_kt(i, res_sb, kt, tiles):
                # one contraction step for both output row-tiles; kt=0 only
                # needs hp=0's normalized rows, so it can run while hp=1's
                # attention stream is still in flight
                for mt in range(KT):
                    if kt == 0:
                        tiles[mt] = psg.tile([P, ISZ], f32, tag="gen", name="gout")
                    nc.tensor.matmul(
                        tiles[mt],
                        lhsT=wout_sb[:, kt, mt * P : (mt + 1) * P].bitcast(f32r),
                        rhs=res_sb[:, kt, i * ISZ : (i + 1) * ISZ].bitcast(f32r),
                        start=(kt == 0),
                        stop=(kt == KT - 1),
                    )

            def emit_outproj_tail(b, i, x_sb, tiles):
                for mt in range(KT):
                    y_sb = outp.tile([P, ISZ], f32, tag="y", name="y")
                    nc.vector.scalar_tensor_tensor(
                        out=y_sb,
                        in0=tiles[mt],
                        scalar=beff_sb[:, mt : mt + 1],
                        in1=x_sb[:, mt, i * ISZ : (i + 1) * ISZ],
                        op0=ADD,
                        op1=ADD,
                    )
                    nc.sync.dma_start(
                        y_d[b, mt * P : (mt + 1) * P, i * ISZ : (i + 1) * ISZ],
                        y_sb,
                    )

            # software-pipelined emission: next batch's x load early, its
            # qk/v generation just before this batch's last out-projection
            # so the PE has filler work while the final groups normalize.
            x_sbs = {0: emit_xload(0)}
            qk_sbs = {0: emit_qkgen(0, x_sbs[0])}
            v_sbs = {0: emit_vgen(0, x_sbs[0])}
            for b in range(BPC):
                res_sb = resp.tile([P, KT, N], f32, tag="res", name=f"res{b}")
                for i in range(NI):
                    if i == 0 and b + 1 < BPC:
                        x_sbs[b + 1] = emit_xload(b + 1)
                    for hp in range(2):
                        emit_attn_group(i, hp, qk_sbs[b], v_sbs[b], res_sb)
                    if i == NI - 1 and b + 1 < BPC:
                        qk_sbs[b + 1] = emit_qkgen(b + 1, x_sbs[b + 1])
                        v_sbs[b + 1] = emit_vgen(b + 1, x_sbs[b + 1])
                    tiles = [None, None]
                    for kt in range(KT):
                        emit_outproj_kt(i, res_sb, kt, tiles)
                    emit_outproj_tail(b, i, x_sbs[b], tiles)

    nc.compile()
    return nc


def _prep_weights(W_qkv, b_qkv, W_out, b_out):
    """Host-side weight reshuffles (cheap, [256, 768]-sized)."""
    Wr = np.ascontiguousarray(W_qkv, dtype=np.float32).reshape(C, NH, 3, DK)
    br = np.ascontiguousarray(b_qkv, dtype=np.float32).reshape(NH, 3, DK)
    # feature tiles: [q0|q1], [k0|k1], [q2|q3], [k2|k3]; q pre-scaled
    cols = []
    for hp in range(2):
        cols.append(Wr[:, 2 * hp, 0] * SCALE)
        cols.append(Wr[:, 2 * hp + 1, 0] * SCALE)
        cols.append(Wr[:, 2 * hp, 1])
        cols.append(Wr[:, 2 * hp + 1, 1])
    wqk = np.concatenate(cols, axis=1)  # [C, 512]
    bq = np.stack(
        [
            np.concatenate([br[2 * hp, 0], br[2 * hp + 1, 0]]) * SCALE
            for hp in range(2)
        ],
        axis=1,
    )  # [128, 2]
    wv = np.concatenate([Wr[:, h, 2] for h in range(NH)], axis=1)  # [C, 256]
    bv = np.concatenate([br[h, 2] for h in range(NH)])  # [256]
    W_out = np.ascontiguousarray(W_out, dtype=np.float32)
    b_eff = (b_out + bv @ W_out).astype(np.float32)  # [256]
    beff = b_eff.reshape(KT, P).T.copy()  # [128, 2] col=mt
    return (
        np.ascontiguousarray(wqk, dtype=np.float32),
        np.ascontiguousarray(bq, dtype=np.float32),
        np.ascontiguousarray(wv, dtype=np.float32),
        W_out,
        np.ascontiguousarray(beff, dtype=np.float32),
    )


def kernel(x, W_qkv, b_qkv, W_out, b_out):
    from concourse.bass_utils import run_bass_kernel_spmd

    if "nc" not in _CACHE:
        _CACHE["nc"] = _build_module()
    nc = _CACHE["nc"]

    x = np.ascontiguousarray(np.asarray(x), dtype=np.float32)
    Bx, Cx, Hx, Wx = x.shape
    x3 = x.reshape(Bx, Cx, Hx * Wx)
    wqk, bq, wv, wout, beff = _prep_weights(
        np.asarray(W_qkv), np.asarray(b_qkv), np.asarray(W_out), np.asarray(b_out)
    )

    ones = np.ones((P, DK), dtype=np.float32)
    in_maps = []
    for c in range(N_CORES):
        in_maps.append(
            {
                "x": np.ascontiguousarray(x3[c * BPC : (c + 1) * BPC]),
                "wqk": wqk,
                "wv": wv,
                "wout": wout,
                "bq": bq,
                "beff": beff,
                "ones": ones,
            }
        )

    res = run_bass_kernel_spmd(nc, in_maps, core_ids=list(range(N_CORES)))
    y = np.concatenate([r["y"] for r in res.results], axis=0)  # [16, 256, 1024]
    return y.reshape(Bx, Cx, Hx, Wx).astype(np.float32)


# revision 35
# speedup vs baseline: 48.8849x; 1.0100x over previous
"""Trainium2 Bass kernel for an AttentionBlock (B=16, C=256, N=1024 tokens,
4 heads x d_k=64), data-parallel over batch across 8 NeuronCores.

Layout strategy: all device math runs in "transposed" token-last layout.
x[b] arrives as [C, N] which is exactly xf^T, the natural stationary operand
(lhsT) for every matmul, and the output y^T [C, N] is exactly the layout the
problem wants back ([B, C, H, W]).  No transposes anywhere.

Per batch element, per core:
  qk^T [512, N] = W_qk^T @ xf^T     (q pre-scaled by dk^-0.5, +b_q on copy;
                                     b_k dropped: constant-over-keys terms
                                     cancel in softmax)
  v    [N, 4, 128]                  (tokens on partitions; cols 64..127 = 1.0
                                     so the PV matmul emits the softmax
                                     denominator replicated on rows 64..127)
  S^T  [j, i] per head = k^T.T @ q^T  -- two d_k=64 heads packed in the
                                     128-row PE array via row strips
  P^T = exp(S^T)                    (no max subtraction: |scores| <= ~11,
                                     exp <= ~1.4e4, safe in fp32)
  O^T[128, i] = v_aug.T @ P^T       accumulated over 8 j-tiles; rows 0-63 are
                                     unnormalized O^T, rows 64-127 all carry
                                     the denominator -> 64-lane reciprocal +
                                     one multiply normalizes, no PE involved
  y^T = W_out^T @ res^T + (b_out + b_v @ W_out) + x^T

All matmuls run as float32r (full fp32 storage, 1 cycle/row on the PE vs 4
for plain fp32); producers feeding matmuls tag outputs f32r via bitcast to
satisfy the BIR verifier.
"""

import numpy as np

N_CORES = 8
B, C = 16, 256
N = 1024  # H*W = 32*32
NH, DK = 4, 64
BPC = B // N_CORES  # batch elements per core
P = 128
KT = 2  # C / 128 contraction tiles
ISZ = 512  # i-tile (query) width
NI = N // ISZ  # 2
NJ = N // P  # 8 key tiles
SCALE = DK ** -0.5

_CACHE = {}


def _build_module():
    import concourse.bass as bass  # noqa: F401
    import concourse.mybir as mybir
    import concourse.tile as tile
    from concourse import bacc

    f32 = mybir.dt.float32
    f32r = mybir.dt.float32r
    ADD = mybir.AluOpType.add
    EXP = mybir.ActivationFunctionType.Exp

    nc = bacc.Bacc(
        "TRN2",
        debug=False,
        enable_asserts=False,
        target_bir_lowering=False,
        num_devices=N_CORES,
    )

    x_d = nc.dram_tensor("x", [BPC, C, N], f32, kind="ExternalInput").ap()
    # all weights packed: [p, kt, 0:512]=wqk, [512:768]=wv, [768:1024]=wout
    wall_d = nc.dram_tensor("wall", [P, KT, 4 * P + NH * DK + C], f32,
                            kind="ExternalInput").ap()
    # small constants packed: [p, 0:2]=bq, [2:4]=beff, [4:68]=ones
    sm_d = nc.dram_tensor("sm", [P, 4 + DK], f32, kind="ExternalInput").ap()
    y_d = nc.dram_tensor("y", [BPC, C, N], f32, kind="ExternalOutput").ap()

    with tile.TileContext(nc) as tc:
        with (
            tc.tile_pool(name="const", bufs=1) as const,
            tc.tile_pool(name="xp", bufs=2) as xp,
            tc.tile_pool(name="qkp", bufs=2) as qkp,
            tc.tile_pool(name="vp", bufs=1) as vp,
            tc.tile_pool(name="ptp", bufs=6) as ptp,
            tc.tile_pool(name="resp", bufs=2) as resp,
            tc.tile_pool(name="smp", bufs=4) as smp,
            tc.tile_pool(name="outp", bufs=4) as outp,
            tc.tile_pool(name="psg", bufs=2, space="PSUM") as psg,
            tc.tile_pool(name="pss", bufs=2, space="PSUM") as pss,
            tc.tile_pool(name="psv", bufs=2, space="PSUM") as psv,
        ):
            # --- constants: one packed weight DMA on the SP queue, one
            # packed small-constant DMA on the idle gpsimd queue ---
            sm_sb = const.tile([P, 4 + DK], f32, tag="sm")
            nc.gpsimd.dma_start(sm_sb.bitcast(f32r), sm_d.bitcast(f32r))
            bq_sb = sm_sb[:, 0:2]
            beff_sb = sm_sb[:, 2:4]
            ones_sb = sm_sb[:, 4:]
            wall_sb = const.tile([P, KT, 4 * P + NH * DK + C], f32, tag="wall")
            # wqk first (it gates the first matmuls), then wv+wout
            nc.sync.dma_start(
                wall_sb[:, :, 0 : 4 * P].bitcast(f32r),
                wall_d[:, :, 0 : 4 * P].bitcast(f32r),
            )
            nc.sync.dma_start(
                wall_sb[:, :, 4 * P :].bitcast(f32r),
                wall_d[:, :, 4 * P :].bitcast(f32r),
            )
            wqk_sb = wall_sb[:, :, 0 : 4 * P]
            wv_sb = wall_sb[:, :, 4 * P : 4 * P + NH * DK]
            wout_sb = wall_sb[:, :, 4 * P + NH * DK :]
            # persistent v tiles (one per batch element); ones block
            # [*, jt, h, 64:128] written once via a broadcast DVE copy
            v_tiles = [
                vp.tile([P, NJ, NH, 2 * DK], f32, tag=f"v{vb}", name=f"v{vb}")
                for vb in range(BPC)
            ]

            def emit_xload(b):
                x_sb = xp.tile([P, KT, N], f32, tag="x", name=f"x{b}")
                # split in halves so the first qk/v matmuls start sooner
                for half in range(2):
                    sl = slice(half * (N // 2), (half + 1) * (N // 2))
                    nc.scalar.dma_start(
                        x_sb[:, :, sl].bitcast(f32r),
                        x_d[b]
                        .rearrange("(kt p) n -> p kt n", p=P)[:, :, sl]
                        .bitcast(f32r),
                    )
                return x_sb

            def emit_qkgen(b, x_sb):
                # feature tiles [q01, k01, q23, k23]; i outer so the first
                # half of x unblocks all four tiles
                qk_sb = [
                    qkp.tile([P, N], f32, tag=f"qk{t}", name=f"qk{t}_{b}")
                    for t in range(4)
                ]
                for i in range(NI):
                    for t in range(4):
                        qt = qk_sb[t]
                        ps = psg.tile([P, ISZ], f32, tag="gen", name="gqk")
                        for kt in range(KT):
                            nc.tensor.matmul(
                                ps,
                                lhsT=wqk_sb[:, kt, t * P : (t + 1) * P].bitcast(f32r),
                                rhs=x_sb[:, kt, i * ISZ : (i + 1) * ISZ].bitcast(f32r),
                                start=(kt == 0),
                                stop=(kt == KT - 1),
                            )
                        dst = qt[:, i * ISZ : (i + 1) * ISZ]
                        if t % 2 == 0:  # q tile: add pre-scaled bias
                            hp = t // 2
                            nc.vector.tensor_scalar_add(
                                dst.bitcast(f32r), ps, bq_sb[:, hp : hp + 1]
                            )
                        else:
                            nc.vector.tensor_copy(dst.bitcast(f32r), ps)
                return qk_sb

            def emit_vgen(b, x_sb):
                v_sb = v_tiles[b]
                nc.vector.tensor_copy(
                    v_sb[:, :, :, DK:].bitcast(f32r),
                    ones_sb.rearrange("p (a b d) -> p a b d", a=1, b=1).to_broadcast(
                        [P, NJ, NH, DK]
                    ),
                )
                for jt in range(NJ):
                    ps = psg.tile([P, ISZ], f32, tag="gen", name="gv")
                    psv_view = ps[:, : NH * DK]
                    for kt in range(KT):
                        nc.tensor.matmul(
                            psv_view,
                            lhsT=x_sb[:, kt, jt * P : (jt + 1) * P].bitcast(f32r),
                            rhs=wv_sb[:, kt, :].bitcast(f32r),
                            start=(kt == 0),
                            stop=(kt == KT - 1),
                        )
                    nc.vector.tensor_copy(
                        v_sb[:, jt, :, 0:DK].bitcast(f32r),
                        psv_view.rearrange("p (h d) -> p h d", h=NH),
                    )
                return v_sb

            def emit_attn_group(i, hp, qk_sb, v_sb, res_sb):
                q_t = qk_sb[2 * hp]
                k_t = qk_sb[2 * hp + 1]
                pvs = [
                    psv.tile([P, ISZ], f32, tag="pv", name=f"pv{h}")
                    for h in range(2)
                ]
                for jt in range(NJ):
                    st = pss.tile([P, 2, ISZ], f32, tag="st", name="st")
                    for h in range(2):
                        nc.tensor.matmul(
                            st[:, h, :],
                            lhsT=k_t[
                                h * DK : (h + 1) * DK, jt * P : (jt + 1) * P
                            ].bitcast(f32r),
                            rhs=q_t[
                                h * DK : (h + 1) * DK, i * ISZ : (i + 1) * ISZ
                            ].bitcast(f32r),
                        )
                    pt = ptp.tile([P, 2, ISZ], f32, tag="pt", name="pt")
                    nc.scalar.activation(pt.bitcast(f32r), st, EXP)
                    for h in range(2):
                        nc.tensor.matmul(
                            pvs[h],
                            lhsT=v_sb[:, jt, 2 * hp + h, :].bitcast(f32r),
                            rhs=pt[:, h, :].bitcast(f32r),
                            start=(jt == 0),
                            stop=(jt == NJ - 1),
                        )
                # normalize: denominator replicated on rows 64..127
                for h in range(2):
                    rcp = smp.tile([DK, ISZ], f32, tag=f"rcp{h}", name=f"rcp{h}")
                    nc.vector.reciprocal(rcp, pvs[h][DK : 2 * DK, :])
                    nc.vector.tensor_mul(
                        res_sb[
                            h * DK : (h + 1) * DK, hp, i * ISZ : (i + 1) * ISZ
                        ].bitcast(f32r),
                        pvs[h][0:DK, :],
                        rcp,
                    )

            def emit_outproj_kt(i, res_sb, kt, tiles):
                # one contraction step for both output row-tiles; kt=0 only
                # needs hp=0's normalized rows, so it can run while hp=1's
                # attention stream is still in flight
                for mt in range(KT):
                    if kt == 0:
                        tiles[mt] = psg.tile([P, ISZ], f32, tag="gen", name="gout")
                    nc.tensor.matmul(
                        tiles[mt],
                        lhsT=wout_sb[:, kt, mt * P : (mt + 1) * P].bitcast(f32r),
                        rhs=res_sb[:, kt, i * ISZ : (i + 1) * ISZ].bitcast(f32r),
                        start=(kt == 0),
                        stop=(kt == KT - 1),
                    )

            def emit_outproj_tail(b, i, x_sb, tiles):
                for mt in range(KT):
                    y_sb = outp.tile([P, ISZ], f32, tag="y", name="y")
                    nc.vector.scalar_tensor_tensor(
                        out=y_sb,
                        in0=tiles[mt],
                        scalar=beff_sb[:, mt : mt + 1],
                        in1=x_sb[:, mt, i * ISZ : (i + 1) * ISZ],
                        op0=ADD,
                        op1=ADD,
                    )
                    nc.sync.dma_start(
                        y_d[b, mt * P : (mt + 1) * P, i * ISZ : (i + 1) * ISZ],
                        y_sb,
                    )

            # software-pipelined emission: next batch's x load early, its
            # qk/v generation just before this batch's last out-projection
            # so the PE has filler work while the final groups normalize.
            x_sbs = {0: emit_xload(0)}
            qk_sbs = {0: emit_qkgen(0, x_sbs[0])}
            v_sbs = {0: emit_vgen(0, x_sbs[0])}
            for b in range(BPC):
                res_sb = resp.tile([P, KT, N], f32, tag="res", name=f"res{b}")
                for i in range(NI):
                    if i == 0 and b + 1 < BPC:
                        x_sbs[b + 1] = emit_xload(b + 1)
                    for hp in range(2):
                        emit_attn_group(i, hp, qk_sbs[b], v_sbs[b], res_sb)
                    if i == NI - 1 and b + 1 < BPC:
                        qk_sbs[b + 1] = emit_qkgen(b + 1, x_sbs[b + 1])
                        v_sbs[b + 1] = emit_vgen(b + 1, x_sbs[b + 1])
                    tiles = [None, None]
                    for kt in range(KT):
                        emit_outproj_kt(i, res_sb, kt, tiles)
                    emit_outproj_tail(b, i, x_sbs[b], tiles)

    nc.compile()
    return nc


def _prep_weights(W_qkv, b_qkv, W_out, b_out):
    """Host-side weight reshuffles (cheap, [256, 768]-sized)."""
    Wr = np.ascontiguousarray(W_qkv, dtype=np.float32).reshape(C, NH, 3, DK)
    br = np.ascontiguousarray(b_qkv, dtype=np.float32).reshape(NH, 3, DK)
    # feature tiles: [q0|q1], [k0|k1], [q2|q3], [k2|k3]; q pre-scaled
    cols = []
    for hp in range(2):
        cols.append(Wr[:, 2 * hp, 0] * SCALE)
        cols.append(Wr[:, 2 * hp + 1, 0] * SCALE)
        cols.append(Wr[:, 2 * hp, 1])
        cols.append(Wr[:, 2 * hp + 1, 1])
    wqk = np.concatenate(cols, axis=1)  # [C, 512]
    bq = np.stack(
        [
            np.concatenate([br[2 * hp, 0], br[2 * hp + 1, 0]]) * SCALE
            for hp in range(2)
        ],
        axis=1,
    )  # [128, 2]
    wv = np.concatenate([Wr[:, h, 2] for h in range(NH)], axis=1)  # [C, 256]
    bv = np.concatenate([br[h, 2] for h in range(NH)])  # [256]
    W_out = np.ascontiguousarray(W_out, dtype=np.float32)
    b_eff = (b_out + bv @ W_out).astype(np.float32)  # [256]
    beff = b_eff.reshape(KT, P).T.copy()  # [128, 2] col=mt
    return (
        np.ascontiguousarray(wqk, dtype=np.float32),
        np.ascontiguousarray(bq, dtype=np.float32),
        np.ascontiguousarray(wv, dtype=np.float32),
        W_out,
        np.ascontiguousarray(beff, dtype=np.float32),
    )


def _device_inputs(x3, W_qkv, b_qkv, W_out, b_out):
    wqk, bq, wv, wout, beff = _prep_weights(
        np.asarray(W_qkv), np.asarray(b_qkv), np.asarray(W_out), np.asarray(b_out)
    )
    def to_pkm(w):  # [C, M] -> [P, KT, M] with row kt*P+p on (p, kt)
        return w.reshape(KT, P, -1).transpose(1, 0, 2)

    wall = np.ascontiguousarray(
        np.concatenate([to_pkm(wqk), to_pkm(wv), to_pkm(wout)], axis=2),
        dtype=np.float32,
    )
    sm = np.ascontiguousarray(
        np.concatenate([bq, beff, np.ones((P, DK), np.float32)], axis=1),
        dtype=np.float32,
    )
    in_maps = []
    for c in range(N_CORES):
        in_maps.append(
            {
                "x": np.ascontiguousarray(x3[c * BPC : (c + 1) * BPC]),
                "wall": wall,
                "sm": sm,
            }
        )
    return in_maps


def kernel(x, W_qkv, b_qkv, W_out, b_out):
    from concourse.bass_utils import run_bass_kernel_spmd

    if "nc" not in _CACHE:
        _CACHE["nc"] = _build_module()
    nc = _CACHE["nc"]

    x = np.ascontiguousarray(np.asarray(x), dtype=np.float32)
    Bx, Cx, Hx, Wx = x.shape
    x3 = x.reshape(Bx, Cx, Hx * Wx)
    in_maps = _device_inputs(x3, W_qkv, b_qkv, W_out, b_out)

    res = run_bass_kernel_spmd(nc, in_maps, core_ids=list(range(N_CORES)))
    y = np.concatenate([r["y"] for r in res.results], axis=0)  # [16, 256, 1024]
    return y.reshape(Bx, Cx, Hx, Wx).astype(np.float32)


# revision 41
# speedup vs baseline: 49.5562x; 1.0137x over previous
"""Trainium2 Bass kernel for an AttentionBlock (B=16, C=256, N=1024 tokens,
4 heads x d_k=64), data-parallel over batch across 8 NeuronCores.

Layout strategy: all device math runs in "transposed" token-last layout.
x[b] arrives as [C, N] which is exactly xf^T, the natural stationary operand
(lhsT) for every matmul, and the output y^T [C, N] is exactly the layout the
problem wants back ([B, C, H, W]).  No transposes anywhere.

Per batch element, per core:
  qk^T [512, N] = W_qk^T @ xf^T     (q pre-scaled by dk^-0.5, +b_q on copy;
                                     b_k dropped: constant-over-keys terms
                                     cancel in softmax)
  v    [N, 4, 128]                  (tokens on partitions; cols 64..127 = 1.0
                                     so the PV matmul emits the softmax
                                     denominator replicated on rows 64..127)
  S^T  [j, i] per head = k^T.T @ q^T  -- two d_k=64 heads packed in the
                                     128-row PE array via row strips
  P^T = exp(S^T)                    (no max subtraction: |scores| <= ~11,
                                     exp <= ~1.4e4, safe in fp32)
  O^T[128, i] = v_aug.T @ P^T       accumulated over 8 j-tiles; rows 0-63 are
                                     unnormalized O^T, rows 64-127 all carry
                                     the denominator -> 64-lane reciprocal +
                                     one multiply normalizes, no PE involved
  y^T = W_out^T @ res^T + (b_out + b_v @ W_out) + x^T

All matmuls run as float32r (full fp32 storage, 1 cycle/row on the PE vs 4
for plain fp32); producers feeding matmuls tag outputs f32r via bitcast to
satisfy the BIR verifier.

Scheduling: engines execute their queues in order, so emission order is the
performance lever.  The attention stream is globally software-pipelined with
a one-stage skew (each stage emits S^T + exp for stage k, then the PV of
stage k-1) so the PE never head-of-line blocks on an exp; group completions
(normalize, out-projection, next batch's qk/v generation) are emitted when
the group's last PV retires.  ACT (exp, ~68us busy) is the bottleneck
engine; everything else hides behind it.
"""

import numpy as np

N_CORES = 8
B, C = 16, 256
N = 1024  # H*W = 32*32
NH, DK = 4, 64
BPC = B // N_CORES  # batch elements per core
P = 128
KT = 2  # C / 128 contraction tiles
ISZ = 512  # i-tile (query) width
NI = N // ISZ  # 2
NJ = N // P  # 8 key tiles
SCALE = DK ** -0.5

_CACHE = {}


def _build_module():
    import concourse.bass as bass  # noqa: F401
    import concourse.mybir as mybir
    import concourse.tile as tile
    from concourse import bacc

    f32 = mybir.dt.float32
    f32r = mybir.dt.float32r
    ADD = mybir.AluOpType.add
    EXP = mybir.ActivationFunctionType.Exp

    nc = bacc.Bacc(
        "TRN2",
        debug=False,
        enable_asserts=False,
        target_bir_lowering=False,
        num_devices=N_CORES,
    )

    x_d = nc.dram_tensor("x", [BPC, C, N], f32, kind="ExternalInput").ap()
    # all weights packed: [p, kt, 0:512]=wqk, [512:768]=wv, [768:1024]=wout
    wall_d = nc.dram_tensor("wall", [P, KT, 4 * P + NH * DK + C], f32,
                            kind="ExternalInput").ap()
    # small constants packed: [p, 0:2]=bq, [2:4]=beff, [4:68]=ones
    sm_d = nc.dram_tensor("sm", [P, 4 + DK], f32, kind="ExternalInput").ap()
    y_d = nc.dram_tensor("y", [BPC, C, N], f32, kind="ExternalOutput").ap()

    with tile.TileContext(nc) as tc:
        with (
            tc.tile_pool(name="const", bufs=1) as const,
            tc.tile_pool(name="xp", bufs=2) as xp,
            tc.tile_pool(name="qkp", bufs=2) as qkp,
            tc.tile_pool(name="vp", bufs=1) as vp,
            tc.tile_pool(name="ptp", bufs=6) as ptp,
            tc.tile_pool(name="resp", bufs=2) as resp,
            tc.tile_pool(name="smp", bufs=4) as smp,
            tc.tile_pool(name="outp", bufs=4) as outp,
            tc.tile_pool(name="psg", bufs=2, space="PSUM") as psg,
            tc.tile_pool(name="pss", bufs=2, space="PSUM") as pss,
            tc.tile_pool(name="psv", bufs=2, space="PSUM") as psv,
        ):
            # --- constants: one packed weight DMA on the SP queue, one
            # packed small-constant DMA on the idle gpsimd queue ---
            sm_sb = const.tile([P, 4 + DK], f32, tag="sm")
            nc.gpsimd.dma_start(sm_sb.bitcast(f32r), sm_d.bitcast(f32r))
            bq_sb = sm_sb[:, 0:2]
            beff_sb = sm_sb[:, 2:4]
            ones_sb = sm_sb[:, 4:]
            wall_sb = const.tile([P, KT, 4 * P + NH * DK + C], f32, tag="wall")
            # wqk first (it gates the first matmuls), then wv+wout
            nc.sync.dma_start(
                wall_sb[:, :, 0 : 4 * P].bitcast(f32r),
                wall_d[:, :, 0 : 4 * P].bitcast(f32r),
            )
            nc.sync.dma_start(
                wall_sb[:, :, 4 * P :].bitcast(f32r),
                wall_d[:, :, 4 * P :].bitcast(f32r),
            )
            wqk_sb = wall_sb[:, :, 0 : 4 * P]
            wv_sb = wall_sb[:, :, 4 * P : 4 * P + NH * DK]
            wout_sb = wall_sb[:, :, 4 * P + NH * DK :]
            # persistent v tiles (one per batch element); ones block
            # [*, jt, h, 64:128] written once via a broadcast DVE copy
            v_tiles = [
                vp.tile([P, NJ, NH, 2 * DK], f32, tag=f"v{vb}", name=f"v{vb}")
                for vb in range(BPC)
            ]

            def emit_xload(b):
                x_sb = xp.tile([P, KT, N], f32, tag="x", name=f"x{b}")
                # split in halves so the first qk/v matmuls start sooner
                for half in range(2):
                    sl = slice(half * (N // 2), (half + 1) * (N // 2))
                    nc.scalar.dma_start(
                        x_sb[:, :, sl].bitcast(f32r),
                        x_d[b]
                        .rearrange("(kt p) n -> p kt n", p=P)[:, :, sl]
                        .bitcast(f32r),
                    )
                return x_sb

            def emit_qkgen(b, x_sb):
                # feature tiles [q01, k01, q23, k23]; i outer so the first
                # half of x unblocks all four tiles
                qk_sb = [
                    qkp.tile([P, N], f32, tag=f"qk{t}", name=f"qk{t}_{b}")
                    for t in range(4)
                ]
                for i in range(NI):
                    for t in range(4):
                        qt = qk_sb[t]
                        ps = psg.tile([P, ISZ], f32, tag="gen", name="gqk")
                        for kt in range(KT):
                            nc.tensor.matmul(
                                ps,
                                lhsT=wqk_sb[:, kt, t * P : (t + 1) * P].bitcast(f32r),
                                rhs=x_sb[:, kt, i * ISZ : (i + 1) * ISZ].bitcast(f32r),
                                start=(kt == 0),
                                stop=(kt == KT - 1),
                            )
                        dst = qt[:, i * ISZ : (i + 1) * ISZ]
                        if t % 2 == 0:  # q tile: add pre-scaled bias
                            hp = t // 2
                            nc.vector.tensor_scalar_add(
                                dst.bitcast(f32r), ps, bq_sb[:, hp : hp + 1]
                            )
                        else:
                            nc.vector.tensor_copy(dst.bitcast(f32r), ps)
                return qk_sb

            def emit_vgen(b, x_sb):
                v_sb = v_tiles[b]
                nc.vector.tensor_copy(
                    v_sb[:, :, :, DK:].bitcast(f32r),
                    ones_sb.rearrange("p (a b d) -> p a b d", a=1, b=1).to_broadcast(
                        [P, NJ, NH, DK]
                    ),
                )
                for jt in range(NJ):
                    ps = psg.tile([P, ISZ], f32, tag="gen", name="gv")
                    psv_view = ps[:, : NH * DK]
                    for kt in range(KT):
                        nc.tensor.matmul(
                            psv_view,
                            lhsT=x_sb[:, kt, jt * P : (jt + 1) * P].bitcast(f32r),
                            rhs=wv_sb[:, kt, :].bitcast(f32r),
                            start=(kt == 0),
                            stop=(kt == KT - 1),
                        )
                    nc.vector.tensor_copy(
                        v_sb[:, jt, :, 0:DK].bitcast(f32r),
                        psv_view.rearrange("p (h d) -> p h d", h=NH),
                    )
                return v_sb

            def emit_pv_stage(stg):
                b2, i2, hp2, jt, pvs, v_sb, pt, res_sb = stg
                for h in range(2):
                    nc.tensor.matmul(
                        pvs[h],
                        lhsT=v_sb[:, jt, 2 * hp2 + h, :].bitcast(f32r),
                        rhs=pt[:, h, :].bitcast(f32r),
                        start=(jt == 0),
                        stop=(jt == NJ - 1),
                    )
                if jt == NJ - 1:
                    on_group_complete(b2, i2, hp2, pvs, res_sb)

            def on_group_complete(b, i, hp, pvs, res_sb):
                # normalize: denominator replicated on rows 64..127
                for h in range(2):
                    rcp = smp.tile([DK, ISZ], f32, tag=f"rcp{h}", name=f"rcp{h}")
                    nc.vector.reciprocal(rcp, pvs[h][DK : 2 * DK, :])
                    nc.vector.tensor_mul(
                        res_sb[
                            h * DK : (h + 1) * DK, hp, i * ISZ : (i + 1) * ISZ
                        ].bitcast(f32r),
                        pvs[h][0:DK, :],
                        rcp,
                    )
                if hp == 0 and i == NI - 1 and b + 1 < BPC:
                    # must be traced before any batch b+1 attention group
                    qk_sbs[b + 1] = emit_qkgen(b + 1, x_sbs[b + 1])
                    v_sbs[b + 1] = emit_vgen(b + 1, x_sbs[b + 1])
                if hp == 1:
                    tiles = [None, None]
                    for kt in range(KT):
                        emit_outproj_kt(i, res_sb, kt, tiles)
                    emit_outproj_tail(b, i, x_sbs[b], tiles)

            def emit_outproj_kt(i, res_sb, kt, tiles):
                # one contraction step for both output row-tiles; kt=0 only
                # needs hp=0's normalized rows, so it can run while hp=1's
                # attention stream is still in flight
                for mt in range(KT):
                    if kt == 0:
                        tiles[mt] = psg.tile([P, ISZ], f32, tag="gen", name="gout")
                    nc.tensor.matmul(
                        tiles[mt],
                        lhsT=wout_sb[:, kt, mt * P : (mt + 1) * P].bitcast(f32r),
                        rhs=res_sb[:, kt, i * ISZ : (i + 1) * ISZ].bitcast(f32r),
                        start=(kt == 0),
                        stop=(kt == KT - 1),
                    )

            def emit_outproj_tail(b, i, x_sb, tiles):
                for mt in range(KT):
                    y_sb = outp.tile([P, ISZ], f32, tag="y", name="y")
                    nc.vector.scalar_tensor_tensor(
                        out=y_sb,
                        in0=tiles[mt],
                        scalar=beff_sb[:, mt : mt + 1],
                        in1=x_sb[:, mt, i * ISZ : (i + 1) * ISZ],
                        op0=ADD,
                        op1=ADD,
                    )
                    nc.sync.dma_start(
                        y_d[b, mt * P : (mt + 1) * P, i * ISZ : (i + 1) * ISZ],
                        y_sb,
                    )

            # globally software-pipelined attention: each stage emits
            # S^T + exp for (g, jt) and only then the PV of the previous
            # stage, so the in-order PE queue never stalls on an exp.
            # Group completions (normalize, out-proj, next batch's qk/v
            # generation) ride along when a stage's last PV is emitted.
            x_sbs = {0: emit_xload(0)}
            qk_sbs = {0: emit_qkgen(0, x_sbs[0])}
            v_sbs = {0: emit_vgen(0, x_sbs[0])}
            res_sbs = {}
            op_tiles = {}
            pending = None
            for b in range(BPC):
                res_sbs[b] = resp.tile([P, KT, N], f32, tag="res", name=f"res{b}")
                for i in range(NI):
                    if i == 0 and b + 1 < BPC:
                        x_sbs[b + 1] = emit_xload(b + 1)
                    for hp in range(2):
                        q_t = qk_sbs[b][2 * hp]
                        k_t = qk_sbs[b][2 * hp + 1]
                        pvs = [
                            psv.tile([P, ISZ], f32, tag="pv", name=f"pv{h}")
                            for h in range(2)
                        ]
                        for jt in range(NJ):
                            st = pss.tile([P, 2, ISZ], f32, tag="st", name="st")
                            for h in range(2):
                                nc.tensor.matmul(
                                    st[:, h, :],
                                    lhsT=k_t[
                                        h * DK : (h + 1) * DK,
                                        jt * P : (jt + 1) * P,
                                    ].bitcast(f32r),
                                    rhs=q_t[
                                        h * DK : (h + 1) * DK,
                                        i * ISZ : (i + 1) * ISZ,
                                    ].bitcast(f32r),
                                )
                            pt = ptp.tile([P, 2, ISZ], f32, tag="pt", name="pt")
                            nc.scalar.activation(pt.bitcast(f32r), st, EXP)
                            if pending is not None:
                                emit_pv_stage(pending)
                            pending = (
                                b, i, hp, jt, pvs, v_sbs[b], pt, res_sbs[b]
                            )
            emit_pv_stage(pending)

    nc.compile()
    return nc


def _prep_weights(W_qkv, b_qkv, W_out, b_out):
    """Host-side weight reshuffles (cheap, [256, 768]-sized)."""
    Wr = np.ascontiguousarray(W_qkv, dtype=np.float32).reshape(C, NH, 3, DK)
    br = np.ascontiguousarray(b_qkv, dtype=np.float32).reshape(NH, 3, DK)
    # feature tiles: [q0|q1], [k0|k1], [q2|q3], [k2|k3]; q pre-scaled
    cols = []
    for hp in range(2):
        cols.append(Wr[:, 2 * hp, 0] * SCALE)
        cols.append(Wr[:, 2 * hp + 1, 0] * SCALE)
        cols.append(Wr[:, 2 * hp, 1])
        cols.append(Wr[:, 2 * hp + 1, 1])
    wqk = np.concatenate(cols, axis=1)  # [C, 512]
    bq = np.stack(
        [
            np.concatenate([br[2 * hp, 0], br[2 * hp + 1, 0]]) * SCALE
            for hp in range(2)
        ],
        axis=1,
    )  # [128, 2]
    wv = np.concatenate([Wr[:, h, 2] for h in range(NH)], axis=1)  # [C, 256]
    bv = np.concatenate([br[h, 2] for h in range(NH)])  # [256]
    W_out = np.ascontiguousarray(W_out, dtype=np.float32)
    b_eff = (b_out + bv @ W_out).astype(np.float32)  # [256]
    beff = b_eff.reshape(KT, P).T.copy()  # [128, 2] col=mt
    return (
        np.ascontiguousarray(wqk, dtype=np.float32),
        np.ascontiguousarray(bq, dtype=np.float32),
        np.ascontiguousarray(wv, dtype=np.float32),
        W_out,
        np.ascontiguousarray(beff, dtype=np.float32),
    )


def _device_inputs(x3, W_qkv, b_qkv, W_out, b_out):
    wqk, bq, wv, wout, beff = _prep_weights(
        np.asarray(W_qkv), np.asarray(b_qkv), np.asarray(W_out), np.asarray(b_out)
    )
    def to_pkm(w):  # [C, M] -> [P, KT, M] with row kt*P+p on (p, kt)
        return w.reshape(KT, P, -1).transpose(1, 0, 2)

    wall = np.ascontiguousarray(
        np.concatenate([to_pkm(wqk), to_pkm(wv), to_pkm(wout)], axis=2),
        dtype=np.float32,
    )
    sm = np.ascontiguousarray(
        np.concatenate([bq, beff, np.ones((P, DK), np.float32)], axis=1),
        dtype=np.float32,
    )
    in_maps = []
    for c in range(N_CORES):
        in_maps.append(
            {
                "x": np.ascontiguousarray(x3[c * BPC : (c + 1) * BPC]),
                "wall": wall,
                "sm": sm,
            }
        )
    return in_maps


def kernel(x, W_qkv, b_qkv, W_out, b_out):
    from concourse.bass_utils import run_bass_kernel_spmd

    if "nc" not in _CACHE:
        _CACHE["nc"] = _build_module()
    nc = _CACHE["nc"]

    x = np.ascontiguousarray(np.asarray(x), dtype=np.float32)
    Bx, Cx, Hx, Wx = x.shape
    x3 = x.reshape(Bx, Cx, Hx * Wx)
    in_maps = _device_inputs(x3, W_qkv, b_qkv, W_out, b_out)

    res = run_bass_kernel_spmd(nc, in_maps, core_ids=list(range(N_CORES)))
    y = np.concatenate([r["y"] for r in res.results], axis=0)  # [16, 256, 1024]
    return y.reshape(Bx, Cx, Hx, Wx).astype(np.float32)


# revision 42
# speedup vs baseline: 50.1268x; 1.0115x over previous
"""Trainium2 Bass kernel for an AttentionBlock (B=16, C=256, N=1024 tokens,
4 heads x d_k=64), data-parallel over batch across 8 NeuronCores.

Layout strategy: all device math runs in "transposed" token-last layout.
x[b] arrives as [C, N] which is exactly xf^T, the natural stationary operand
(lhsT) for every matmul, and the output y^T [C, N] is exactly the layout the
problem wants back ([B, C, H, W]).  No transposes anywhere.

Per batch element, per core:
  qk^T [512, N] = W_qk^T @ xf^T     (q pre-scaled by dk^-0.5, +b_q on copy;
                                     b_k dropped: constant-over-keys terms
                                     cancel in softmax)
  v    [N, 4, 128]                  (tokens on partitions; cols 64..127 = 1.0
                                     so the PV matmul emits the softmax
                                     denominator replicated on rows 64..127)
  S^T  [j, i] per head = k^T.T @ q^T  -- two d_k=64 heads packed in the
                                     128-row PE array via row strips
  P^T = exp(S^T)                    (no max subtraction: |scores| <= ~11,
                                     exp <= ~1.4e4, safe in fp32)
  O^T[128, i] = v_aug.T @ P^T       accumulated over 8 j-tiles; rows 0-63 are
                                     unnormalized O^T, rows 64-127 all carry
                                     the denominator -> 64-lane reciprocal +
                                     one multiply normalizes, no PE involved
  y^T = W_out^T @ res^T + (b_out + b_v @ W_out) + x^T

All matmuls run as float32r (full fp32 storage, 1 cycle/row on the PE vs 4
for plain fp32); producers feeding matmuls tag outputs f32r via bitcast to
satisfy the BIR verifier.

Scheduling: engines execute their queues in order, so emission order is the
performance lever.  The attention stream is globally software-pipelined with
a one-stage skew (each stage emits S^T + exp for stage k, then the PV of
stage k-1) so the PE never head-of-line blocks on an exp; group completions
(normalize, out-projection, next batch's qk/v generation) are emitted when
the group's last PV retires.  ACT (exp, ~68us busy) is the bottleneck
engine; everything else hides behind it.
"""

import numpy as np

N_CORES = 8
B, C = 16, 256
N = 1024  # H*W = 32*32
NH, DK = 4, 64
BPC = B // N_CORES  # batch elements per core
P = 128
KT = 2  # C / 128 contraction tiles
ISZ = 512  # i-tile (query) width
NI = N // ISZ  # 2
NJ = N // P  # 8 key tiles
SCALE = DK ** -0.5

_CACHE = {}


def _build_module():
    import concourse.bass as bass  # noqa: F401
    import concourse.mybir as mybir
    import concourse.tile as tile
    from concourse import bacc

    f32 = mybir.dt.float32
    f32r = mybir.dt.float32r
    ADD = mybir.AluOpType.add
    EXP = mybir.ActivationFunctionType.Exp

    nc = bacc.Bacc(
        "TRN2",
        debug=False,
        enable_asserts=False,
        target_bir_lowering=False,
        num_devices=N_CORES,
    )

    x_d = nc.dram_tensor("x", [BPC, C, N], f32, kind="ExternalInput").ap()
    # all weights packed: [p, kt, 0:512]=wqk, [512:768]=wv, [768:1024]=wout
    wall_d = nc.dram_tensor("wall", [P, KT, 4 * P + NH * DK + C], f32,
                            kind="ExternalInput").ap()
    # small constants packed: [p, 0:2]=bq, [2:4]=beff, [4:68]=ones
    sm_d = nc.dram_tensor("sm", [P, 4 + DK], f32, kind="ExternalInput").ap()
    y_d = nc.dram_tensor("y", [BPC, C, N], f32, kind="ExternalOutput").ap()

    with tile.TileContext(nc) as tc:
        with (
            tc.tile_pool(name="const", bufs=1) as const,
            tc.tile_pool(name="xp", bufs=2) as xp,
            tc.tile_pool(name="qkp", bufs=2) as qkp,
            tc.tile_pool(name="vp", bufs=1) as vp,
            tc.tile_pool(name="ptp", bufs=6) as ptp,
            tc.tile_pool(name="resp", bufs=2) as resp,
            tc.tile_pool(name="smp", bufs=4) as smp,
            tc.tile_pool(name="outp", bufs=4) as outp,
            tc.tile_pool(name="psg", bufs=2, space="PSUM") as psg,
            tc.tile_pool(name="pss", bufs=2, space="PSUM") as pss,
            tc.tile_pool(name="psv", bufs=2, space="PSUM") as psv,
        ):
            # --- constants: one packed weight DMA on the SP queue, one
            # packed small-constant DMA on the idle gpsimd queue ---
            sm_sb = const.tile([P, 4 + DK], f32, tag="sm")
            nc.gpsimd.dma_start(sm_sb.bitcast(f32r), sm_d.bitcast(f32r))
            bq_sb = sm_sb[:, 0:2]
            beff_sb = sm_sb[:, 2:4]
            ones_sb = sm_sb[:, 4:]
            wall_sb = const.tile([P, KT, 4 * P + NH * DK + C], f32, tag="wall")
            # wqk first (it gates the first matmuls), then wv+wout
            nc.sync.dma_start(
                wall_sb[:, :, 0 : 4 * P].bitcast(f32r),
                wall_d[:, :, 0 : 4 * P].bitcast(f32r),
            )
            nc.sync.dma_start(
                wall_sb[:, :, 4 * P :].bitcast(f32r),
                wall_d[:, :, 4 * P :].bitcast(f32r),
            )
            wqk_sb = wall_sb[:, :, 0 : 4 * P]
            wv_sb = wall_sb[:, :, 4 * P : 4 * P + NH * DK]
            wout_sb = wall_sb[:, :, 4 * P + NH * DK :]
            # persistent v tiles (one per batch element); ones block
            # [*, jt, h, 64:128] written once via a broadcast DVE copy
            v_tiles = [
                vp.tile([P, NJ, NH, 2 * DK], f32, tag=f"v{vb}", name=f"v{vb}")
                for vb in range(BPC)
            ]

            def emit_xload(b):
                x_sb = xp.tile([P, KT, N], f32, tag="x", name=f"x{b}")
                # split in halves so the first qk/v matmuls start sooner
                for half in range(2):
                    sl = slice(half * (N // 2), (half + 1) * (N // 2))
                    nc.scalar.dma_start(
                        x_sb[:, :, sl].bitcast(f32r),
                        x_d[b]
                        .rearrange("(kt p) n -> p kt n", p=P)[:, :, sl]
                        .bitcast(f32r),
                    )
                return x_sb

            def alloc_qk(b):
                return [
                    qkp.tile([P, N], f32, tag=f"qk{t}", name=f"qk{t}_{b}")
                    for t in range(4)
                ]

            def emit_qkgen(b, x_sb, qk_sb=None, i_list=None, t_list=None):
                # feature tiles [q01, k01, q23, k23]; i outer so the first
                # half of x unblocks all four tiles
                if qk_sb is None:
                    qk_sb = alloc_qk(b)
                for i in (range(NI) if i_list is None else i_list):
                    for t in (range(4) if t_list is None else t_list):
                        qt = qk_sb[t]
                        ps = psg.tile([P, ISZ], f32, tag="gen", name="gqk")
                        for kt in range(KT):
                            nc.tensor.matmul(
                                ps,
                                lhsT=wqk_sb[:, kt, t * P : (t + 1) * P].bitcast(f32r),
                                rhs=x_sb[:, kt, i * ISZ : (i + 1) * ISZ].bitcast(f32r),
                                start=(kt == 0),
                                stop=(kt == KT - 1),
                            )
                        dst = qt[:, i * ISZ : (i + 1) * ISZ]
                        if t % 2 == 0:  # q tile: add pre-scaled bias
                            hp = t // 2
                            nc.vector.tensor_scalar_add(
                                dst.bitcast(f32r), ps, bq_sb[:, hp : hp + 1]
                            )
                        else:
                            nc.vector.tensor_copy(dst.bitcast(f32r), ps)
                return qk_sb

            def emit_vgen(b, x_sb, jt_list=None, ones=True):
                v_sb = v_tiles[b]
                if ones:
                    nc.vector.tensor_copy(
                        v_sb[:, :, :, DK:].bitcast(f32r),
                        ones_sb.rearrange(
                            "p (a b d) -> p a b d", a=1, b=1
                        ).to_broadcast([P, NJ, NH, DK]),
                    )
                for jt in (range(NJ) if jt_list is None else jt_list):
                    ps = psg.tile([P, ISZ], f32, tag="gen", name="gv")
                    psv_view = ps[:, : NH * DK]
                    for kt in range(KT):
                        nc.tensor.matmul(
                            psv_view,
                            lhsT=x_sb[:, kt, jt * P : (jt + 1) * P].bitcast(f32r),
                            rhs=wv_sb[:, kt, :].bitcast(f32r),
                            start=(kt == 0),
                            stop=(kt == KT - 1),
                        )
                    nc.vector.tensor_copy(
                        v_sb[:, jt, :, 0:DK].bitcast(f32r),
                        psv_view.rearrange("p (h d) -> p h d", h=NH),
                    )
                return v_sb

            def emit_pv_stage(stg):
                b2, i2, hp2, jt, pvs, v_sb, pt, res_sb = stg
                for h in range(2):
                    nc.tensor.matmul(
                        pvs[h],
                        lhsT=v_sb[:, jt, 2 * hp2 + h, :].bitcast(f32r),
                        rhs=pt[:, h, :].bitcast(f32r),
                        start=(jt == 0),
                        stop=(jt == NJ - 1),
                    )
                if jt == NJ - 1:
                    on_group_complete(b2, i2, hp2, pvs, res_sb)

            def on_group_complete(b, i, hp, pvs, res_sb):
                # normalize: denominator replicated on rows 64..127
                for h in range(2):
                    rcp = smp.tile([DK, ISZ], f32, tag=f"rcp{h}", name=f"rcp{h}")
                    nc.vector.reciprocal(rcp, pvs[h][DK : 2 * DK, :])
                    nc.vector.tensor_mul(
                        res_sb[
                            h * DK : (h + 1) * DK, hp, i * ISZ : (i + 1) * ISZ
                        ].bitcast(f32r),
                        pvs[h][0:DK, :],
                        rcp,
                    )
                if hp == 0 and i == NI - 1 and b + 1 < BPC:
                    # must be traced before any batch b+1 attention group
                    qk_sbs[b + 1] = emit_qkgen(b + 1, x_sbs[b + 1])
                    v_sbs[b + 1] = emit_vgen(b + 1, x_sbs[b + 1])
                if hp == 1:
                    tiles = [None, None]
                    for kt in range(KT):
                        emit_outproj_kt(i, res_sb, kt, tiles)
                    emit_outproj_tail(b, i, x_sbs[b], tiles)

            def emit_outproj_kt(i, res_sb, kt, tiles):
                # one contraction step for both output row-tiles; kt=0 only
                # needs hp=0's normalized rows, so it can run while hp=1's
                # attention stream is still in flight
                for mt in range(KT):
                    if kt == 0:
                        tiles[mt] = psg.tile([P, ISZ], f32, tag="gen", name="gout")
                    nc.tensor.matmul(
                        tiles[mt],
                        lhsT=wout_sb[:, kt, mt * P : (mt + 1) * P].bitcast(f32r),
                        rhs=res_sb[:, kt, i * ISZ : (i + 1) * ISZ].bitcast(f32r),
                        start=(kt == 0),
                        stop=(kt == KT - 1),
                    )

            def emit_outproj_tail(b, i, x_sb, tiles):
                for mt in range(KT):
                    y_sb = outp.tile([P, ISZ], f32, tag="y", name="y")
                    nc.vector.scalar_tensor_tensor(
                        out=y_sb,
                        in0=tiles[mt],
                        scalar=beff_sb[:, mt : mt + 1],
                        in1=x_sb[:, mt, i * ISZ : (i + 1) * ISZ],
                        op0=ADD,
                        op1=ADD,
                    )
                    nc.sync.dma_start(
                        y_d[b, mt * P : (mt + 1) * P, i * ISZ : (i + 1) * ISZ],
                        y_sb,
                    )

            # globally software-pipelined attention: each stage emits
            # S^T + exp for (g, jt) and only then the PV of the previous
            # stage, so the in-order PE queue never stalls on an exp.
            # Group completions (normalize, out-proj, next batch's qk/v
            # generation) ride along when a stage's last PV is emitted.
            x_sbs = {0: emit_xload(0)}
            # minimal prologue: just q01/k01 on the first half of x and the
            # first four v tiles, so exp starts as early as possible; the
            # rest of batch 0's generation is injected into the first
            # attention group's stream (see below).
            qk_sbs = {0: emit_qkgen(0, x_sbs[0], i_list=[0], t_list=[0, 1])}
            v_sbs = {0: emit_vgen(0, x_sbs[0], jt_list=[0, 1, 2, 3])}
            res_sbs = {}
            op_tiles = {}
            pending = None
            deferred_gen = True
            for b in range(BPC):
                res_sbs[b] = resp.tile([P, KT, N], f32, tag="res", name=f"res{b}")
                for i in range(NI):
                    if i == 0 and b + 1 < BPC:
                        x_sbs[b + 1] = emit_xload(b + 1)
                    for hp in range(2):
                        q_t = qk_sbs[b][2 * hp]
                        k_t = qk_sbs[b][2 * hp + 1]
                        pvs = [
                            psv.tile([P, ISZ], f32, tag="pv", name=f"pv{h}")
                            for h in range(2)
                        ]
                        for jt in range(NJ):
                            st = pss.tile([P, 2, ISZ], f32, tag="st", name="st")
                            for h in range(2):
                                nc.tensor.matmul(
                                    st[:, h, :],
                                    lhsT=k_t[
                                        h * DK : (h + 1) * DK,
                                        jt * P : (jt + 1) * P,
                                    ].bitcast(f32r),
                                    rhs=q_t[
                                        h * DK : (h + 1) * DK,
                                        i * ISZ : (i + 1) * ISZ,
                                    ].bitcast(f32r),
                                )
                            pt = ptp.tile([P, 2, ISZ], f32, tag="pt", name="pt")
                            nc.scalar.activation(pt.bitcast(f32r), st, EXP)
                            if pending is not None:
                                emit_pv_stage(pending)
                            pending = (
                                b, i, hp, jt, pvs, v_sbs[b], pt, res_sbs[b]
                            )
                            if deferred_gen and jt == 1:
                                # rest of batch 0's generation, traced before
                                # any stage that consumes it (j>=4 needs the
                                # second halves; hp=1 needs q23/k23)
                                deferred_gen = False
                                emit_qkgen(0, x_sbs[0], qk_sb=qk_sbs[0],
                                           i_list=[0], t_list=[2, 3])
                                emit_qkgen(0, x_sbs[0], qk_sb=qk_sbs[0],
                                           i_list=[1])
                                emit_vgen(0, x_sbs[0], jt_list=[4, 5, 6, 7],
                                          ones=False)
            emit_pv_stage(pending)

    nc.compile()
    return nc


def _prep_weights(W_qkv, b_qkv, W_out, b_out):
    """Host-side weight reshuffles (cheap, [256, 768]-sized)."""
    Wr = np.ascontiguousarray(W_qkv, dtype=np.float32).reshape(C, NH, 3, DK)
    br = np.ascontiguousarray(b_qkv, dtype=np.float32).reshape(NH, 3, DK)
    # feature tiles: [q0|q1], [k0|k1], [q2|q3], [k2|k3]; q pre-scaled
    cols = []
    for hp in range(2):
        cols.append(Wr[:, 2 * hp, 0] * SCALE)
        cols.append(Wr[:, 2 * hp + 1, 0] * SCALE)
        cols.append(Wr[:, 2 * hp, 1])
        cols.append(Wr[:, 2 * hp + 1, 1])
    wqk = np.concatenate(cols, axis=1)  # [C, 512]
    bq = np.stack(
        [
            np.concatenate([br[2 * hp, 0], br[2 * hp + 1, 0]]) * SCALE
            for hp in range(2)
        ],
        axis=1,
    )  # [128, 2]
    wv = np.concatenate([Wr[:, h, 2] for h in range(NH)], axis=1)  # [C, 256]
    bv = np.concatenate([br[h, 2] for h in range(NH)])  # [256]
    W_out = np.ascontiguousarray(W_out, dtype=np.float32)
    b_eff = (b_out + bv @ W_out).astype(np.float32)  # [256]
    beff = b_eff.reshape(KT, P).T.copy()  # [128, 2] col=mt
    return (
        np.ascontiguousarray(wqk, dtype=np.float32),
        np.ascontiguousarray(bq, dtype=np.float32),
        np.ascontiguousarray(wv, dtype=np.float32),
        W_out,
        np.ascontiguousarray(beff, dtype=np.float32),
    )


def _device_inputs(x3, W_qkv, b_qkv, W_out, b_out):
    wqk, bq, wv, wout, beff = _prep_weights(
        np.asarray(W_qkv), np.asarray(b_qkv), np.asarray(W_out), np.asarray(b_out)
    )
    def to_pkm(w):  # [C, M] -> [P, KT, M] with row kt*P+p on (p, kt)
        return w.reshape(KT, P, -1).transpose(1, 0, 2)

    wall = np.ascontiguousarray(
        np.concatenate([to_pkm(wqk), to_pkm(wv), to_pkm(wout)], axis=2),
        dtype=np.float32,
    )
    sm = np.ascontiguousarray(
        np.concatenate([bq, beff, np.ones((P, DK), np.float32)], axis=1),
        dtype=np.float32,
    )
    in_maps = []
    for c in range(N_CORES):
        in_maps.append(
            {
                "x": np.ascontiguousarray(x3[c * BPC : (c + 1) * BPC]),
                "wall": wall,
                "sm": sm,
            }
        )
    return in_maps


def kernel(x, W_qkv, b_qkv, W_out, b_out):
    from concourse.bass_utils import run_bass_kernel_spmd

    if "nc" not in _CACHE:
        _CACHE["nc"] = _build_module()
    nc = _CACHE["nc"]

    x = np.ascontiguousarray(np.asarray(x), dtype=np.float32)
    Bx, Cx, Hx, Wx = x.shape
    x3 = x.reshape(Bx, Cx, Hx * Wx)
    in_maps = _device_inputs(x3, W_qkv, b_qkv, W_out, b_out)

    res = run_bass_kernel_spmd(nc, in_maps, core_ids=list(range(N_CORES)))
    y = np.concatenate([r["y"] for r in res.results], axis=0)  # [16, 256, 1024]
    return y.reshape(Bx, Cx, Hx, Wx).astype(np.float32)
